# revision 11
# baseline (speedup 1.0000x reference)
"""Deformable cross-attention kernel for 8 Trainium2 NeuronCores.

Data-parallel over batch N=8: core i processes batch element i.
Per-core pipeline (v3):
  1. qT (host pre-transposed) -> offsets/attn projection (fp32 matmul)
  2. DVE weight math: corner coefficients with edge remapping, softmax,
     4-tap coefficient products P[xj][yj] = c_xj * attn*wy_yj, gather
     entry indices idx = clip(y0)*128 + clip(x0)
  3. memT (host pre-transposed, bf16) -> bf16 matmul with W_value ->
     value rows drained into a y-duplicated DRAM layout
     value_d[pair][entry=(y0,x)][dy*64+c] so ONE 512B gather window
     (entries x0, x0+1) holds all 4 bilinear corners of a sample
  4. dma_gather: 32 calls (head, point) x 1024 idxs, spread across
     SWDGE queues 1..3 (async desc-gen on distinct Q7 CPU pairs) + 0
  5. DVE blend: 4 taps/sample, accumulate over points, + sfac*b_value
  6. PE transpose of sampled, f32r output projection
"""
import json
import numpy as np
import ml_dtypes

N_B = 8
LQ = 1024
C = 256
NH = 8
NPT = 4
HD = 32
HW = 16384
GRID = 128  # H == W == 128
ENT = GRID * GRID  # entries (y0, x) per head-pair
EPAD = 2  # zero pad entries at end of each pair


def _patch_compat(bass):
    """Split multi-wait instructions and sem-range-clears for this walrus."""
    if getattr(bass.Bass, "_dca_patched", False):
        return
    orig = bass.Bass.to_json_bytes

    def to_json_bytes(self):
        m = json.loads(orig(self))
        uid = 0
        sem_names = m.get("ant_sem_names") or {}
        for fn in m["functions"]:
            for bb in fn["blocks"]:
                out = []
                for inst in bb["instructions"]:
                    si = inst.get("sync_info")
                    waits = (si or {}).get("on_wait") or []
                    if len(waits) > 1:
                        for w in waits[:-1]:
                            uid += 1
                            out.append({
                                "debug": inst.get("debug", 0),
                                "engine": inst["engine"],
                                "ins": [], "outs": [],
                                "name": f"I-wsplit-{uid}",
                                "opcode": "EventSemaphore",
                                "sync_info": {"on_update": [], "on_wait": [w]},
                            })
                        si["on_wait"] = waits[-1:]
                    if (inst.get("opcode") == "ISA"
                            and inst.get("op_name") == "EVENT_SEMAPHORE_RANGE_CLEAR"):
                        d = inst["ant_dict"]
                        for sid in range(d["range_first"], d["range_last"] + 1):
                            uid += 1
                            out.append({
                                "debug": inst.get("debug", 0),
                                "engine": inst["engine"],
                                "ins": [], "outs": [],
                                "name": f"I-semclr-{uid}",
                                "opcode": "EventSemaphore",
                                "sync_info": {
                                    "on_wait": [],
                                    "on_update": [{
                                        "ant_name": sem_names.get(str(sid), f"sem{sid}"),
                                        "id": sid, "sync_type": "semaphore",
                                        "update_mode": "sem-wr-imm",
                                        "update_value": 0,
                                    }]},
                            })
                        continue
                    out.append(inst)
                bb["instructions"] = out
        return json.dumps(m).encode()

    bass.Bass.to_json_bytes = to_json_bytes
    bass.Bass._dca_patched = True


def _floor(nc, pool, mybir, x, shape, pfx):
    """Exact floor via double cast + is_gt correction (any cast rounding)."""
    F32 = mybir.dt.float32
    xi = pool.tile(shape, mybir.dt.int32, name=f"{pfx}_xi", tag=f"{pfx}_xi")
    nc.vector.tensor_copy(xi[:], x[:])
    xf = pool.tile(shape, F32, name=f"{pfx}_xf", tag=f"{pfx}_xf")
    nc.vector.tensor_copy(xf[:], xi[:])
    gt = pool.tile(shape, F32, name=f"{pfx}_gt", tag=f"{pfx}_gt")
    nc.vector.tensor_tensor(gt[:], xf[:], x[:], mybir.AluOpType.is_gt)
    fl = pool.tile(shape, F32, name=f"{pfx}_fl", tag=f"{pfx}_fl")
    nc.vector.tensor_tensor(fl[:], xf[:], gt[:], mybir.AluOpType.subtract)
    return fl


def build_program():
    import concourse.bass as bass
    import concourse.bacc as bacc
    import concourse.mybir as mybir
    import concourse.tile as tile
    from contextlib import ExitStack

    _patch_compat(bass)

    F32 = mybir.dt.float32
    F32R = mybir.dt.float32r
    BF16 = mybir.dt.bfloat16
    I16 = mybir.dt.int16
    TT = mybir.AluOpType
    ACTF = mybir.ActivationFunctionType

    nc = bacc.Bacc(num_swdge_queues=4)

    # ---- external tensors ----
    qT_e = nc.declare_dram_parameter("qT", [C, LQ], F32, isOutput=False)
    memT_e = nc.declare_dram_parameter("memT", [C, HW], BF16, isOutput=False)
    refpts_e = nc.declare_dram_parameter("refpts", [LQ, 2], F32, isOutput=False)
    w_value_e = nc.declare_dram_parameter("w_value", [C, C], BF16, isOutput=False)
    w_oa_e = nc.declare_dram_parameter("w_oa", [C, 96], F32, isOutput=False)
    b_oa_e = nc.declare_dram_parameter("b_oa", [128, 96], F32, isOutput=False)
    w_out_e = nc.declare_dram_parameter("w_out", [C, C], F32R, isOutput=False)
    b_out_e = nc.declare_dram_parameter("b_out", [128, C], F32, isOutput=False)
    b_val_e = nc.declare_dram_parameter("b_val", [128, C], F32, isOutput=False)
    ident_e = nc.declare_dram_parameter("ident", [128, 128], F32, isOutput=False)
    out_e = nc.declare_dram_parameter("out", [LQ, C], F32, isOutput=True)

    # internal DRAM: value entries [4 pairs][(y0, x)][dy*64 + c] bf16
    value_d = nc.dram_tensor("value_ydup", [4, ENT + EPAD, 128], BF16)

    with tile.TileContext(nc) as tc, ExitStack() as ctx:
        cpool = ctx.enter_context(tc.tile_pool(name="const", bufs=1))
        qpool = ctx.enter_context(tc.tile_pool(name="qp", bufs=2))
        mpool = ctx.enter_context(tc.tile_pool(name="mem", bufs=3))
        spool = ctx.enter_context(tc.tile_pool(name="stg", bufs=3))
        gpool = ctx.enter_context(tc.tile_pool(name="gat", bufs=6))
        wm = ctx.enter_context(tc.tile_pool(name="wmath", bufs=1))
        psA = ctx.enter_context(tc.tile_pool(name="psA", bufs=2, space="PSUM"))
        psB = ctx.enter_context(tc.tile_pool(name="psB", bufs=2, space="PSUM"))
        psV = ctx.enter_context(tc.tile_pool(name="psV", bufs=4, space="PSUM"))
        _n = [0]

        def ps_tr():  # [128,128] f32 transpose target
            _n[0] += 1
            return psA.tile([128, 128], F32, name=f"pstr{_n[0]}", tag="pstr")

        def ps_mm():  # [128,256] f32 matmul target
            _n[0] += 1
            return psB.tile([128, C], F32, name=f"psmm{_n[0]}", tag="psmm")

        def ps_v():  # [128,256] f32 value matmul target
            _n[0] += 1
            return psV.tile([128, C], F32, name=f"psv{_n[0]}", tag="psv")

        # value weights first so the value pipeline can start ASAP
        w_val = cpool.tile([128, 2, C], BF16)
        nc.sync.dma_start(w_val[:], w_value_e[:].rearrange(
            "(k p) o -> p k o", k=2))
        ident = cpool.tile([128, 128], F32)
        nc.sync.dma_start(ident[:], ident_e[:])

        # ---------- 1. queryT (host pre-transposed) + projections ----------
        qTt = cpool.tile([128, 2, LQ], F32)
        nc.sync.dma_start(qTt[:], qT_e[:].rearrange("(k p) q -> p k q", k=2))
        qT = [qTt[:, 0, :], qTt[:, 1, :]]

        w_oa = cpool.tile([128, 2, 96], F32)
        nc.sync.dma_start(w_oa[:], w_oa_e[:].rearrange("(k p) o -> p k o", k=2))
        b_oa = cpool.tile([128, 96], F32)
        nc.sync.dma_start(b_oa[:], b_oa_e[:])

        # off_all [128, 8, 96] (partition = q%128, qtop free)
        off_all = cpool.tile([128, 8, 96], F32)
        for qc in range(8):
            po = ps_mm()
            for kc in range(2):
                nc.tensor.matmul(po[:, 0:96], qT[kc][:, qc * 128:(qc + 1) * 128],
                                 w_oa[:, kc, :], start=(kc == 0), stop=(kc == 1))
            nc.vector.tensor_tensor(off_all[:, qc, :], po[:, 0:96], b_oa[:], TT.add)

        refs = cpool.tile([128, 8, 2], F32)
        nc.sync.dma_start(
            refs[:], refpts_e[:].rearrange("(g p) t -> p g t", p=128))
        w_out = cpool.tile([128, 2, C], F32R)
        nc.scalar.dma_start(w_out[:], w_out_e[:].rearrange(
            "(k p) o -> p k o", k=2))
        b_out = cpool.tile([128, C], F32)
        nc.scalar.dma_start(b_out[:], b_out_e[:])
        b_val = cpool.tile([128, C], F32)
        nc.scalar.dma_start(b_val[:], b_val_e[:])

        # ---------- 2. weight math (emitted interleaved with value loop) ----
        S8 = [128, 8, 32]      # (q%128, qtop, (h, pt))

        def view_off(comp):  # comp 0 = x, 1 = y -> [128, 8, 8, 4] strided view
            return off_all[:, :, comp:64 + comp].rearrange(
                "p g (h pt two) -> p g h pt two", h=8, two=2)[:, :, :, :, 0]

        _wmn = [0]

        def ttile():
            _wmn[0] += 1
            nm = f"wm{_wmn[0]}"
            return wm.tile(S8, F32, name=nm, tag=nm)

        def h4(x):
            return x[:].rearrange("p g (h pt) -> p g h pt", pt=4)

        P4 = cpool.tile([128, 8, 2, 2, 8, 4], F32)
        sfac = cpool.tile([128, 8, 8], F32)
        idxf = cpool.tile([128, 8, 8, 4], F32)
        tall = [cpool.tile([128, 128], F32, name=f"tall{t}", tag=f"tall{t}")
                for t in range(2)]
        idx_f2 = cpool.tile([16, 32, 8, 8], F32)  # [m, call=(h,pt), qtop, qmid]
        idx16 = cpool.tile([16, 32 * 64], I16)
        idxr = cpool.tile([128, 32, 64], I16)

        def gfloor(x, pfx):
            xi = wm.tile(S8, mybir.dt.int32, name=f"{pfx}_xi", tag=f"{pfx}_xi")
            nc.vector.tensor_copy(xi[:], x[:])
            yield
            xf = wm.tile(S8, F32, name=f"{pfx}_xf", tag=f"{pfx}_xf")
            nc.vector.tensor_copy(xf[:], xi[:])
            yield
            gt = wm.tile(S8, F32, name=f"{pfx}_gt", tag=f"{pfx}_gt")
            nc.vector.tensor_tensor(gt[:], xf[:], x[:], mybir.AluOpType.is_gt)
            yield
            fl = wm.tile(S8, F32, name=f"{pfx}_fl", tag=f"{pfx}_fl")
            nc.vector.tensor_tensor(fl[:], xf[:], gt[:], mybir.AluOpType.subtract)
            yield fl

        def corner_chain(comp, pfx):
            """x/y 1-D chain -> (cA, cB, clamped) with edge remapping."""
            p_ = ttile()
            nc.vector.tensor_scalar(p_[:], view_off(comp), 1.0 / GRID, None,
                                    TT.mult)
            yield
            nc.vector.tensor_tensor(
                p_[:], p_[:], refs[:, :, comp:comp + 1].broadcast_to(S8), TT.add)
            yield
            nc.vector.tensor_scalar(p_[:], p_[:], float(GRID), -0.5, TT.mult,
                                    TT.add)
            yield
            z0 = None
            for z0 in gfloor(p_, pfx):
                yield
            w1 = ttile()
            nc.vector.tensor_tensor(w1[:], p_[:], z0[:], TT.subtract)
            yield
            w0 = ttile()
            nc.vector.tensor_scalar(w0[:], w1[:], -1.0, 1.0, TT.mult, TT.add)
            yield
            ge0 = ttile()
            nc.vector.tensor_scalar(ge0[:], z0[:], 0.0, None, TT.is_ge)
            yield
            le127 = ttile()
            nc.vector.tensor_scalar(le127[:], z0[:], 127.0, None, TT.is_le)
            yield
            le126 = ttile()
            nc.vector.tensor_scalar(le126[:], z0[:], 126.0, None, TT.is_le)
            yield
            eqm1 = ttile()
            nc.vector.tensor_scalar(eqm1[:], z0[:], -1.0, None, TT.is_equal)
            yield
            # cA = w0*inb(z0) + w1*(z0 == -1);  cB = w1*(0 <= z0 <= 126)
            cA = ttile()
            nc.vector.tensor_tensor(cA[:], ge0[:], le127[:], TT.mult)
            yield
            nc.vector.tensor_tensor(cA[:], cA[:], w0[:], TT.mult)
            yield
            t_ = ttile()
            nc.vector.tensor_tensor(t_[:], w1[:], eqm1[:], TT.mult)
            yield
            nc.vector.tensor_tensor(cA[:], cA[:], t_[:], TT.add)
            yield
            cB = ttile()
            nc.vector.tensor_tensor(cB[:], ge0[:], le126[:], TT.mult)
            yield
            nc.vector.tensor_tensor(cB[:], cB[:], w1[:], TT.mult)
            yield
            zs = ttile()
            nc.vector.tensor_scalar(zs[:], z0[:], 0.0, 127.0, TT.max, TT.min)
            yield (cA, cB, zs)

        def wm_gen():
            xres = yres = None
            for xres in corner_chain(0, "fx"):
                yield
            c0, c1, xs = xres
            for yres in corner_chain(1, "fy"):
                yield
            d0, d1, ys = yres
            # softmax over pt
            logit4 = off_all[:, :, 64:96].rearrange(
                "p g (h pt) -> p g h pt", pt=4)
            mx = wm.tile([128, 8, 8], F32, name="smx", tag="smx")
            nc.vector.tensor_reduce(mx[:], logit4, mybir.AxisListType.X, TT.max)
            yield
            ee = ttile()
            nc.vector.tensor_tensor(
                h4(ee), logit4,
                mx[:].unsqueeze(3).broadcast_to([128, 8, 8, 4]), TT.subtract)
            yield
            nc.scalar.activation(ee[:], ee[:], ACTF.Exp)
            yield
            ssum = wm.tile([128, 8, 8], F32, name="ssum", tag="ssum")
            nc.vector.tensor_reduce(ssum[:], h4(ee), mybir.AxisListType.X,
                                    TT.add)
            yield
            rec = wm.tile([128, 8, 8], F32, name="srec", tag="srec")
            nc.vector.reciprocal(rec[:], ssum[:])
            yield
            attn = ttile()
            nc.vector.tensor_tensor(
                h4(attn), h4(ee),
                rec[:].unsqueeze(3).broadcast_to([128, 8, 8, 4]), TT.mult)
            yield
            # 4-tap coefficients P[xj][yj] = c_xj * (attn * d_yj)
            g0 = ttile()
            nc.vector.tensor_tensor(g0[:], attn[:], d0[:], TT.mult)
            yield
            g1 = ttile()
            nc.vector.tensor_tensor(g1[:], attn[:], d1[:], TT.mult)
            yield
            for xj, cx in ((0, c0), (1, c1)):
                for yj, gy in ((0, g0), (1, g1)):
                    nc.vector.tensor_tensor(
                        P4[:, :, xj, yj, :, :], h4(cx), h4(gy), TT.mult)
                    yield
            # bias-fold factor S[q, h] = sum_pt (c0+c1)*(g0+g1)
            wys = ttile()
            nc.vector.tensor_tensor(wys[:], g0[:], g1[:], TT.add)
            yield
            cxs = ttile()
            nc.vector.tensor_tensor(cxs[:], c0[:], c1[:], TT.add)
            yield
            nc.vector.tensor_tensor(wys[:], wys[:], cxs[:], TT.mult)
            yield
            nc.vector.tensor_reduce(sfac[:], h4(wys), mybir.AxisListType.X,
                                    TT.add)
            yield
            # gather entry indices idxf = ys*128 + xs
            nc.vector.scalar_tensor_tensor(
                idxf[:], h4(ys), float(GRID), h4(xs), TT.mult, TT.add)
            yield
            # idx layout transform: [128, 256] -> wrapped [128, 32, 64]
            idxf_flat = idxf[:].rearrange("p g h pt -> p (g h pt)")
            for t in range(2):
                pt_ = ps_tr()
                nc.tensor.transpose(
                    pt_[:], idxf_flat[:, t * 128:(t + 1) * 128], ident[:])
                nc.vector.tensor_copy(tall[t][:], pt_[:])
                yield
            for t in range(2):
                for qmid in range(8):
                    ptf = ps_tr()
                    pt_ = ptf[0:16, :]
                    nc.tensor.transpose(
                        pt_, tall[t][:, qmid * 16:qmid * 16 + 16], ident[:])
                    # f' = (qtop%4)*32 + call; qtop = t*4 + f'//32
                    srcv = pt_.rearrange("a (q4 c) -> a q4 c", q4=4)
                    nc.vector.tensor_copy(
                        idx_f2[:, :, 4 * t:4 * t + 4, qmid].rearrange(
                            "a c q4 -> a q4 c"), srcv)
                    yield
            nc.vector.tensor_copy(
                idx16[:], idx_f2[:].rearrange("a c g q -> a (c g q)"))
            yield
            for rep in range(8):
                nc.gpsimd.dma_start(
                    idxr[rep * 16:(rep + 1) * 16, :, :],
                    idx16[:].rearrange("a (c b) -> a c b", c=32))
            yield

        # ---------- 3. value projection + y-duplicated store ----------
        zpad = cpool.tile([4, 128], BF16)
        nc.vector.memset(zpad[:], 0.0)
        for pr in range(4):
            nc.gpsimd.dma_start(value_d[pr, ENT:ENT + EPAD, :],
                                zpad[:].rearrange("(a b) c -> a b c", a=2)[pr % 2])

        # stage tiles: [128 x, 4 pr, 8 rows, 2 dy, 64 c] bf16 per 8-row batch
        def stg_tile(b):
            return spool.tile([128, 4, 8, 2, 64], BF16, name=f"stg{b}",
                              tag="stg")

        def flush(b, st):
            for pr in range(4):
                eng = (nc.sync, nc.scalar, nc.sync, nc.scalar)[pr]
                eng.dma_start(
                    value_d[pr, b * 1024:(b + 1) * 1024, :].rearrange(
                        "(y x) l -> x y l", x=128),
                    st[:, pr].rearrange("x y d c -> x y (d c)"))

        wmg = wm_gen()
        stages = {}
        memt = None
        for r in range(GRID):  # row r: positions r*128 .. r*128+127
            if r % 8 == 0:
                memt = mpool.tile([128, 2, 1024], BF16, name="memt", tag="memt")
                nc.sync.dma_start(
                    memt[:],
                    memT_e[:, r * 128:(r + 8) * 128].rearrange(
                        "(k p) q -> p k q", k=2))
                stages[r // 8] = stg_tile(r // 8)
            pv = ps_v()
            for kc in range(2):
                nc.tensor.matmul(
                    pv[:], memt[:, kc, (r % 8) * 128:(r % 8) * 128 + 128],
                    w_val[:, kc, :], start=(kc == 0), stop=(kc == 1))
            # dy=0 slot of row r from psum (ACT); dy=1 slot of row r-1 is a
            # cheap bf16 SBUF shift-copy of row r's dy=0 slot (DVE 2x mode)
            pv4 = pv[:].rearrange("p (a c) -> p a c", a=4)
            nc.scalar.copy(stages[r // 8][:, :, r % 8, 0, :], pv4)
            if r > 0:
                nc.vector.tensor_copy(
                    stages[(r - 1) // 8][:, :, (r - 1) % 8, 1, :],
                    stages[r // 8][:, :, r % 8, 0, :])
            next(wmg, None)  # interleave one weight-math DVE op
            if r % 8 == 0 and r > 0:
                b = r // 8 - 1
                flush(b, stages.pop(b))
        for _ in wmg:  # drain remaining weight-math ops
            pass
        nc.vector.memset(stages[15][:, :, 7, 1, :], 0.0)
        flush(15, stages.pop(15))

        # ---------- 4 & 5. gather + blend ----------
        sampled = cpool.tile([128, 8, 8, 32], F32)  # [q%128, qtop, h, c]
        bterm = cpool.tile([128, 8, 4, 32], F32)
        sT = [cpool.tile([128, 8, 128], F32R, name=f"sTi{i}", tag=f"sTi{i}")
              for i in range(2)]

        def half_tail(hf):
            # bias fold for heads 4hf..4hf+3, then transpose into sT[hf]
            sl = sampled[:, :, hf * 4:(hf + 1) * 4, :]
            nc.vector.tensor_tensor(
                bterm[:],
                sfac[:, :, hf * 4:(hf + 1) * 4].unsqueeze(3).broadcast_to(
                    [128, 8, 4, 32]),
                b_val[:].rearrange("p (h c) -> p h c", h=8)[
                    :, hf * 4:(hf + 1) * 4, :].unsqueeze(1).broadcast_to(
                    [128, 8, 4, 32]),
                TT.mult)
            nc.vector.tensor_tensor(sl, sl, bterm[:], TT.add)
            for qt_ in range(8):
                pt_ = ps_tr()
                nc.tensor.transpose(
                    pt_[:], sl[:, qt_].rearrange("p h c -> p (h c)"), ident[:])
                nc.scalar.copy(sT[hf][:, qt_, :], pt_[:])

        val_flat = value_d[:].rearrange("pr r c -> (pr r c)")
        QSCHED = (1, 2, 3, 0)
        for h in range(NH):
            pr = h // 2
            half = h % 2
            base = pr * ((ENT + EPAD) * 128)
            in_ap = val_flat[base:base + ENT * 128].rearrange(
                "(n c) -> n c", c=128).copy()
            in_ap.ap[-1] = (1, 256)  # overlapping 256-elem windows, step 128
            for pt_i in range(NPT):
                call = h * NPT + pt_i
                gat = gpool.tile([128, 8, 256], BF16)
                nc.gpsimd.dma_gather(
                    gat[:], in_ap, idxr[:, call, :], LQ, LQ, 256,
                    elem_step=128, queue_num=QSCHED[call % 4])
                # taps: lane = xj*256 + dy*64 + half*32 + c
                gv = gat[:].rearrange("p g (xj dy hl c) -> p g xj dy hl c",
                                      xj=2, dy=2, hl=2)
                acc = gpool.tile([128, 8, 2, 32], F32, tag="acc")
                tmp = gpool.tile([128, 8, 2, 32], F32, tag="tmp")
                cf0 = P4[:, :, 0, :, h, pt_i].unsqueeze(3).broadcast_to(
                    [128, 8, 2, 32])
                cf1 = P4[:, :, 1, :, h, pt_i].unsqueeze(3).broadcast_to(
                    [128, 8, 2, 32])
                nc.vector.tensor_tensor(acc[:], gv[:, :, 0, :, half, :], cf0,
                                        TT.mult)
                nc.vector.tensor_tensor(tmp[:], gv[:, :, 1, :, half, :], cf1,
                                        TT.mult)
                nc.vector.tensor_tensor(acc[:], acc[:], tmp[:], TT.add)
                if pt_i == 0:
                    nc.vector.tensor_reduce(
                        sampled[:, :, h, :],
                        acc[:].rearrange("p g d c -> p g c d"),
                        mybir.AxisListType.X, TT.add)
                else:
                    red = gpool.tile([128, 8, 32], F32, tag="red")
                    nc.vector.tensor_reduce(
                        red[:], acc[:].rearrange("p g d c -> p g c d"),
                        mybir.AxisListType.X, TT.add)
                    nc.vector.tensor_tensor(sampled[:, :, h, :],
                                            sampled[:, :, h, :], red[:], TT.add)
            if h == 3 or h == 7:
                half_tail(h // 4)

        # ---------- 6. output projection (sT halves emitted per h-half) ----
        for qt_ in range(8):
            po = ps_mm()
            for kc in range(2):
                nc.tensor.matmul(po[:], sT[kc][:, qt_, :], w_out[:, kc, :],
                                 start=(kc == 0), stop=(kc == 1))
            ot = qpool.tile([128, C], F32, tag="out")
            nc.vector.tensor_tensor(ot[:], po[:], b_out[:], TT.add)
            nc.sync.dma_start(out_e[qt_ * 128:(qt_ + 1) * 128, :], ot[:])

    nc.finalize()
    return nc


_CACHE = {}


def _get_program():
    if "nc" not in _CACHE:
        _CACHE["nc"] = build_program()
    return _CACHE["nc"]


def run(inputs, trace=False):
    from concourse.bass_utils import run_bass_kernel_spmd

    nc = _get_program()
    query = np.asarray(inputs["query"], np.float32)
    memory = np.asarray(inputs["memory"], np.float32)
    refpts = np.asarray(inputs["reference_points"], np.float32)
    w_value = np.asarray(inputs["W_value"], np.float32).astype(ml_dtypes.bfloat16)
    b_value = np.asarray(inputs["b_value"], np.float32)
    w_off = np.asarray(inputs["W_off"], np.float32)
    b_off = np.asarray(inputs["b_off"], np.float32)
    w_attn = np.asarray(inputs["W_attn"], np.float32)
    b_attn = np.asarray(inputs["b_attn"], np.float32)
    w_out = np.asarray(inputs["W_out"], np.float32)
    b_out = np.asarray(inputs["b_out"], np.float32)

    w_oa = np.concatenate([w_off, w_attn], axis=1).astype(np.float32)
    b_oa = np.tile(np.concatenate([b_off, b_attn])[None, :], (128, 1)).astype(
        np.float32)
    b_out_r = np.tile(b_out[None, :], (128, 1)).astype(np.float32)
    b_val_r = np.tile(b_value[None, :], (128, 1)).astype(np.float32)
    ident = np.eye(128, dtype=np.float32)

    shared = dict(w_value=w_value, w_oa=w_oa, b_oa=b_oa, w_out=w_out,
                  b_out=b_out_r, b_val=b_val_r, ident=ident)
    in_maps = []
    for i in range(N_B):
        m = dict(shared)
        m["qT"] = np.ascontiguousarray(query[i].T)
        m["memT"] = np.ascontiguousarray(memory[i].T).astype(ml_dtypes.bfloat16)
        m["refpts"] = refpts[i]
        in_maps.append(m)

    res = run_bass_kernel_spmd(nc, in_maps, list(range(N_B)), trace=trace,
                               trace_cores=[0])
    out = np.stack([res.results[i]["out"] for i in range(N_B)], axis=0)
    return out, res


def kernel(**inputs):
    assert int(inputs.get("H", GRID)) == GRID and int(inputs.get("W", GRID)) == GRID
    out, _ = run(inputs, trace=False)
    return out.astype(np.float32)


# revision 17
# speedup vs baseline: 1.0435x; 1.0435x over previous
"""Deformable cross-attention kernel for 8 Trainium2 NeuronCores.

Data-parallel over batch N=8: core i processes batch element i.
Per-core pipeline (v3):
  1. qT (host pre-transposed) -> offsets/attn projection (fp32 matmul)
  2. DVE weight math: corner coefficients with edge remapping, softmax,
     4-tap coefficient products P[xj][yj] = c_xj * attn*wy_yj, gather
     entry indices idx = clip(y0)*128 + clip(x0)
  3. memT (host pre-transposed, bf16) -> bf16 matmul with W_value ->
     value rows drained into a y-duplicated DRAM layout
     value_d[pair][entry=(y0,x)][dy*64+c] so ONE 512B gather window
     (entries x0, x0+1) holds all 4 bilinear corners of a sample
  4. dma_gather: 32 calls (head, point) x 1024 idxs, spread across
     SWDGE queues 1..3 (async desc-gen on distinct Q7 CPU pairs) + 0
  5. DVE blend: 4 taps/sample, accumulate over points, + sfac*b_value
  6. PE transpose of sampled, f32r output projection
"""
import json
import numpy as np
import ml_dtypes

N_B = 8
LQ = 1024
C = 256
NH = 8
NPT = 4
HD = 32
HW = 16384
GRID = 128  # H == W == 128
ENT = GRID * GRID  # entries (y0, x) per head-pair
EPAD = 2  # zero pad entries at end of each pair


def _patch_compat(bass):
    """Split multi-wait instructions and sem-range-clears for this walrus."""
    if getattr(bass.Bass, "_dca_patched", False):
        return
    orig = bass.Bass.to_json_bytes

    def to_json_bytes(self):
        m = json.loads(orig(self))
        uid = 0
        sem_names = m.get("ant_sem_names") or {}
        for fn in m["functions"]:
            for bb in fn["blocks"]:
                out = []
                for inst in bb["instructions"]:
                    si = inst.get("sync_info")
                    waits = (si or {}).get("on_wait") or []
                    if len(waits) > 1:
                        for w in waits[:-1]:
                            uid += 1
                            out.append({
                                "debug": inst.get("debug", 0),
                                "engine": inst["engine"],
                                "ins": [], "outs": [],
                                "name": f"I-wsplit-{uid}",
                                "opcode": "EventSemaphore",
                                "sync_info": {"on_update": [], "on_wait": [w]},
                            })
                        si["on_wait"] = waits[-1:]
                    if (inst.get("opcode") == "ISA"
                            and inst.get("op_name") == "EVENT_SEMAPHORE_RANGE_CLEAR"):
                        d = inst["ant_dict"]
                        for sid in range(d["range_first"], d["range_last"] + 1):
                            uid += 1
                            out.append({
                                "debug": inst.get("debug", 0),
                                "engine": inst["engine"],
                                "ins": [], "outs": [],
                                "name": f"I-semclr-{uid}",
                                "opcode": "EventSemaphore",
                                "sync_info": {
                                    "on_wait": [],
                                    "on_update": [{
                                        "ant_name": sem_names.get(str(sid), f"sem{sid}"),
                                        "id": sid, "sync_type": "semaphore",
                                        "update_mode": "sem-wr-imm",
                                        "update_value": 0,
                                    }]},
                            })
                        continue
                    out.append(inst)
                bb["instructions"] = out
        return json.dumps(m).encode()

    bass.Bass.to_json_bytes = to_json_bytes
    bass.Bass._dca_patched = True


def _floor(nc, pool, mybir, x, shape, pfx):
    """Exact floor via double cast + is_gt correction (any cast rounding)."""
    F32 = mybir.dt.float32
    xi = pool.tile(shape, mybir.dt.int32, name=f"{pfx}_xi", tag=f"{pfx}_xi")
    nc.vector.tensor_copy(xi[:], x[:])
    xf = pool.tile(shape, F32, name=f"{pfx}_xf", tag=f"{pfx}_xf")
    nc.vector.tensor_copy(xf[:], xi[:])
    gt = pool.tile(shape, F32, name=f"{pfx}_gt", tag=f"{pfx}_gt")
    nc.vector.tensor_tensor(gt[:], xf[:], x[:], mybir.AluOpType.is_gt)
    fl = pool.tile(shape, F32, name=f"{pfx}_fl", tag=f"{pfx}_fl")
    nc.vector.tensor_tensor(fl[:], xf[:], gt[:], mybir.AluOpType.subtract)
    return fl


def build_program():
    import concourse.bass as bass
    import concourse.bacc as bacc
    import concourse.mybir as mybir
    import concourse.tile as tile
    from contextlib import ExitStack

    _patch_compat(bass)

    F32 = mybir.dt.float32
    F32R = mybir.dt.float32r
    BF16 = mybir.dt.bfloat16
    I16 = mybir.dt.int16
    TT = mybir.AluOpType
    ACTF = mybir.ActivationFunctionType

    nc = bacc.Bacc(num_swdge_queues=4)

    # ---- external tensors ----
    qT_e = nc.declare_dram_parameter("qT", [C, LQ], F32, isOutput=False)
    memT_e = nc.declare_dram_parameter("memT", [C, HW], BF16, isOutput=False)
    refpts_e = nc.declare_dram_parameter("refpts", [LQ, 2], F32, isOutput=False)
    w_value_e = nc.declare_dram_parameter("w_value", [C, C], BF16, isOutput=False)
    w_oa_e = nc.declare_dram_parameter("w_oa", [C, 96], F32, isOutput=False)
    b_oa_e = nc.declare_dram_parameter("b_oa", [128, 96], F32, isOutput=False)
    w_out_e = nc.declare_dram_parameter("w_out", [C, C], F32R, isOutput=False)
    b_out_e = nc.declare_dram_parameter("b_out", [128, C], F32, isOutput=False)
    b_val_e = nc.declare_dram_parameter("b_val", [128, C], F32, isOutput=False)
    ident_e = nc.declare_dram_parameter("ident", [128, 128], F32, isOutput=False)
    out_e = nc.declare_dram_parameter("out", [LQ, C], F32, isOutput=True)

    # internal DRAM: value entries [4 pairs][(y0, x)][dy*64 + c] bf16
    value_d = nc.dram_tensor("value_ydup", [4, ENT + EPAD, 128], BF16)

    with tile.TileContext(nc) as tc, ExitStack() as ctx:
        cpool = ctx.enter_context(tc.tile_pool(name="const", bufs=1))
        qpool = ctx.enter_context(tc.tile_pool(name="qp", bufs=2))
        mpool = ctx.enter_context(tc.tile_pool(name="mem", bufs=3))
        spool = ctx.enter_context(tc.tile_pool(name="stg", bufs=3))
        gpool = ctx.enter_context(tc.tile_pool(name="gat", bufs=1))
        bpool = ctx.enter_context(tc.tile_pool(name="bl", bufs=2))
        wm = ctx.enter_context(tc.tile_pool(name="wmath", bufs=1))
        psA = ctx.enter_context(tc.tile_pool(name="psA", bufs=2, space="PSUM"))
        psB = ctx.enter_context(tc.tile_pool(name="psB", bufs=2, space="PSUM"))
        psV = ctx.enter_context(tc.tile_pool(name="psV", bufs=4, space="PSUM"))
        _n = [0]

        def ps_tr():  # [128,128] f32 transpose target
            _n[0] += 1
            return psA.tile([128, 128], F32, name=f"pstr{_n[0]}", tag="pstr")

        def ps_mm():  # [128,256] f32 matmul target
            _n[0] += 1
            return psB.tile([128, C], F32, name=f"psmm{_n[0]}", tag="psmm")

        def ps_v():  # [128,256] f32 value matmul target
            _n[0] += 1
            return psV.tile([128, C], F32, name=f"psv{_n[0]}", tag="psv")

        # value weights first so the value pipeline can start ASAP
        w_val = cpool.tile([128, 2, C], BF16)
        nc.sync.dma_start(w_val[:], w_value_e[:].rearrange(
            "(k p) o -> p k o", k=2))
        ident = cpool.tile([128, 128], F32)
        nc.sync.dma_start(ident[:], ident_e[:])

        # ---------- 1. queryT (host pre-transposed) + projections ----------
        qTt = cpool.tile([128, 2, LQ], F32)
        nc.sync.dma_start(qTt[:], qT_e[:].rearrange("(k p) q -> p k q", k=2))
        qT = [qTt[:, 0, :], qTt[:, 1, :]]

        w_oa = cpool.tile([128, 2, 96], F32)
        nc.sync.dma_start(w_oa[:], w_oa_e[:].rearrange("(k p) o -> p k o", k=2))
        b_oa = cpool.tile([128, 96], F32)
        nc.sync.dma_start(b_oa[:], b_oa_e[:])

        # off_all [128, 8, 96] (partition = q%128, qtop free)
        off_all = cpool.tile([128, 8, 96], F32)
        for qc in range(8):
            po = ps_mm()
            for kc in range(2):
                nc.tensor.matmul(po[:, 0:96], qT[kc][:, qc * 128:(qc + 1) * 128],
                                 w_oa[:, kc, :], start=(kc == 0), stop=(kc == 1))
            nc.vector.tensor_tensor(off_all[:, qc, :], po[:, 0:96], b_oa[:], TT.add)

        refs = cpool.tile([128, 8, 2], F32)
        nc.sync.dma_start(
            refs[:], refpts_e[:].rearrange("(g p) t -> p g t", p=128))
        w_out = cpool.tile([128, 2, C], F32R)
        nc.scalar.dma_start(w_out[:], w_out_e[:].rearrange(
            "(k p) o -> p k o", k=2))
        b_out = cpool.tile([128, C], F32)
        nc.scalar.dma_start(b_out[:], b_out_e[:])
        b_val = cpool.tile([128, C], F32)
        nc.scalar.dma_start(b_val[:], b_val_e[:])

        # ---------- 2. weight math (emitted interleaved with value loop) ----
        S8 = [128, 8, 32]      # (q%128, qtop, (h, pt))

        def view_off(comp):  # comp 0 = x, 1 = y -> [128, 8, 8, 4] strided view
            return off_all[:, :, comp:64 + comp].rearrange(
                "p g (h pt two) -> p g h pt two", h=8, two=2)[:, :, :, :, 0]

        _wmn = [0]

        def ttile():
            _wmn[0] += 1
            nm = f"wm{_wmn[0]}"
            return wm.tile(S8, F32, name=nm, tag=nm)

        def h4(x):
            return x[:].rearrange("p g (h pt) -> p g h pt", pt=4)

        P4c = cpool.tile([128, 2, 2, 8, 4, 8], F32)
        sfac = cpool.tile([128, 8, 8], F32)
        idxf = cpool.tile([128, 8, 8, 4], F32)
        tall = [cpool.tile([128, 128], F32, name=f"tall{t}", tag=f"tall{t}")
                for t in range(2)]
        idx_f2 = cpool.tile([16, 32, 8, 8], I16)  # [m, call=(h,pt), qtop, qmid]
        idxr = cpool.tile([128, 16, 128], I16)

        def gfloor(x, pfx):
            xi = wm.tile(S8, mybir.dt.int32, name=f"{pfx}_xi", tag="cc_xi")
            nc.vector.tensor_copy(xi[:], x[:])
            yield
            xf = wm.tile(S8, F32, name=f"{pfx}_xf", tag="cc_xf")
            nc.vector.tensor_copy(xf[:], xi[:])
            yield
            gt = wm.tile(S8, F32, name=f"{pfx}_gt", tag="cc_gt")
            nc.vector.tensor_tensor(gt[:], xf[:], x[:], mybir.AluOpType.is_gt)
            yield
            fl = wm.tile(S8, F32, name=f"{pfx}_fl", tag="cc_fl")
            nc.vector.tensor_tensor(fl[:], xf[:], gt[:], mybir.AluOpType.subtract)
            yield fl

        def cctile(nm):
            return wm.tile(S8, F32, name=f"cc_{nm}", tag=f"cc_{nm}")

        def corner_chain(comp, pfx):
            """x/y 1-D chain -> (cA, cB, clamped) with edge remapping."""
            p_ = cctile("p")
            nc.vector.tensor_scalar(p_[:], view_off(comp), 1.0 / GRID, None,
                                    TT.mult)
            yield
            nc.vector.tensor_tensor(
                p_[:], p_[:], refs[:, :, comp:comp + 1].broadcast_to(S8), TT.add)
            yield
            nc.vector.tensor_scalar(p_[:], p_[:], float(GRID), -0.5, TT.mult,
                                    TT.add)
            yield
            z0 = None
            for z0 in gfloor(p_, pfx):
                yield
            w1 = cctile("w1")
            nc.vector.tensor_tensor(w1[:], p_[:], z0[:], TT.subtract)
            yield
            w0 = cctile("w0")
            nc.vector.tensor_scalar(w0[:], w1[:], -1.0, 1.0, TT.mult, TT.add)
            yield
            ge0 = cctile("ge0")
            nc.vector.tensor_scalar(ge0[:], z0[:], 0.0, None, TT.is_ge)
            yield
            le127 = cctile("le127")
            nc.vector.tensor_scalar(le127[:], z0[:], 127.0, None, TT.is_le)
            yield
            le126 = cctile("le126")
            nc.vector.tensor_scalar(le126[:], z0[:], 126.0, None, TT.is_le)
            yield
            eqm1 = cctile("eqm1")
            nc.vector.tensor_scalar(eqm1[:], z0[:], -1.0, None, TT.is_equal)
            yield
            # cA = w0*inb(z0) + w1*(z0 == -1);  cB = w1*(0 <= z0 <= 126)
            cA = wm.tile(S8, F32, name=f"{pfx}_cA", tag=f"{pfx}_cA")
            nc.vector.tensor_tensor(cA[:], ge0[:], le127[:], TT.mult)
            yield
            nc.vector.tensor_tensor(cA[:], cA[:], w0[:], TT.mult)
            yield
            t_ = cctile("t")
            nc.vector.tensor_tensor(t_[:], w1[:], eqm1[:], TT.mult)
            yield
            nc.vector.tensor_tensor(cA[:], cA[:], t_[:], TT.add)
            yield
            cB = wm.tile(S8, F32, name=f"{pfx}_cB", tag=f"{pfx}_cB")
            nc.vector.tensor_tensor(cB[:], ge0[:], le126[:], TT.mult)
            yield
            nc.vector.tensor_tensor(cB[:], cB[:], w1[:], TT.mult)
            yield
            zs = wm.tile(S8, F32, name=f"{pfx}_zs", tag=f"{pfx}_zs")
            nc.vector.tensor_scalar(zs[:], z0[:], 0.0, 127.0, TT.max, TT.min)
            yield (cA, cB, zs)

        def wm_gen():
            xres = yres = None
            for xres in corner_chain(0, "fx"):
                yield
            c0, c1, xs = xres
            for yres in corner_chain(1, "fy"):
                yield
            d0, d1, ys = yres
            # softmax over pt
            logit4 = off_all[:, :, 64:96].rearrange(
                "p g (h pt) -> p g h pt", pt=4)
            mx = wm.tile([128, 8, 8], F32, name="smx", tag="smx")
            nc.vector.tensor_reduce(mx[:], logit4, mybir.AxisListType.X, TT.max)
            yield
            ee = ttile()
            nc.vector.tensor_tensor(
                h4(ee), logit4,
                mx[:].unsqueeze(3).broadcast_to([128, 8, 8, 4]), TT.subtract)
            yield
            nc.scalar.activation(ee[:], ee[:], ACTF.Exp)
            yield
            ssum = wm.tile([128, 8, 8], F32, name="ssum", tag="ssum")
            nc.vector.tensor_reduce(ssum[:], h4(ee), mybir.AxisListType.X,
                                    TT.add)
            yield
            rec = wm.tile([128, 8, 8], F32, name="srec", tag="srec")
            nc.vector.reciprocal(rec[:], ssum[:])
            yield
            attn = ttile()
            nc.vector.tensor_tensor(
                h4(attn), h4(ee),
                rec[:].unsqueeze(3).broadcast_to([128, 8, 8, 4]), TT.mult)
            yield
            # 4-tap coefficients P[xj][yj] = c_xj * (attn * d_yj)
            g0 = ttile()
            nc.vector.tensor_tensor(g0[:], attn[:], d0[:], TT.mult)
            yield
            g1 = ttile()
            nc.vector.tensor_tensor(g1[:], attn[:], d1[:], TT.mult)
            yield
            for xj, cx in ((0, c0), (1, c1)):
                for yj, gy in ((0, g0), (1, g1)):
                    nc.vector.tensor_tensor(
                        P4c[:, xj, yj].rearrange("p h pt g -> p g h pt"),
                        h4(cx), h4(gy), TT.mult)
                    yield
            # bias-fold factor S[q, h] = sum_pt (c0+c1)*(g0+g1)
            wys = ttile()
            nc.vector.tensor_tensor(wys[:], g0[:], g1[:], TT.add)
            yield
            cxs = ttile()
            nc.vector.tensor_tensor(cxs[:], c0[:], c1[:], TT.add)
            yield
            nc.vector.tensor_tensor(wys[:], wys[:], cxs[:], TT.mult)
            yield
            nc.vector.tensor_reduce(sfac[:], h4(wys), mybir.AxisListType.X,
                                    TT.add)
            yield
            # gather entry indices idxf = ys*128 + xs
            nc.vector.scalar_tensor_tensor(
                idxf[:], h4(ys), float(GRID), h4(xs), TT.mult, TT.add)
            yield
            # idx layout transform: [128, 256] -> wrapped [128, 32, 64]
            idxf_flat = idxf[:].rearrange("p g h pt -> p (g h pt)")
            for t in range(2):
                pt_ = ps_tr()
                nc.tensor.transpose(
                    pt_[:], idxf_flat[:, t * 128:(t + 1) * 128], ident[:])
                nc.vector.tensor_copy(tall[t][:], pt_[:])
                yield
            for t in range(2):
                for qmid in range(8):
                    ptf = ps_tr()
                    pt_ = ptf[0:16, :]
                    nc.tensor.transpose(
                        pt_, tall[t][:, qmid * 16:qmid * 16 + 16], ident[:])
                    # f' = (qtop%4)*32 + call; qtop = t*4 + f'//32
                    srcv = pt_.rearrange("a (q4 c) -> a q4 c", q4=4)
                    nc.vector.tensor_copy(
                        idx_f2[:, :, 4 * t:4 * t + 4, qmid].rearrange(
                            "a c q4 -> a q4 c"), srcv)
                    yield
            for rep in range(8):
                nc.gpsimd.dma_start(
                    idxr[rep * 16:(rep + 1) * 16, :, :],
                    idx_f2[:].rearrange("a c g q -> a (c g q)").rearrange(
                        "a (c b) -> a c b", c=16))
            yield

        # ---------- 3. value projection + y-duplicated store ----------
        zpad = cpool.tile([4, 128], BF16)
        nc.vector.memset(zpad[:], 0.0)
        for pr in range(4):
            nc.gpsimd.dma_start(value_d[pr, ENT:ENT + EPAD, :],
                                zpad[:].rearrange("(a b) c -> a b c", a=2)[pr % 2])

        # stage tiles: [128 x, 4 pr, 8 rows, 2 dy, 64 c] bf16 per 8-row batch
        def stg_tile(b):
            return spool.tile([128, 4, 8, 2, 64], BF16, name=f"stg{b}",
                              tag="stg")

        def flush(b, st):
            for pr in range(4):
                eng = (nc.sync, nc.scalar, nc.sync, nc.scalar)[pr]
                eng.dma_start(
                    value_d[pr, b * 1024:(b + 1) * 1024, :].rearrange(
                        "(y x) l -> x y l", x=128),
                    st[:, pr].rearrange("x y d c -> x y (d c)"))

        wmg = wm_gen()
        stages = {}
        memt = None
        for r in range(GRID):  # row r: positions r*128 .. r*128+127
            if r % 8 == 0:
                memt = mpool.tile([128, 2, 1024], BF16, name="memt", tag="memt")
                nc.sync.dma_start(
                    memt[:],
                    memT_e[:, r * 128:(r + 8) * 128].rearrange(
                        "(k p) q -> p k q", k=2))
                stages[r // 8] = stg_tile(r // 8)
            pv = ps_v()
            for kc in range(2):
                nc.tensor.matmul(
                    pv[:], memt[:, kc, (r % 8) * 128:(r % 8) * 128 + 128],
                    w_val[:, kc, :], start=(kc == 0), stop=(kc == 1))
            # dy=0 slot of row r from psum (ACT); dy=1 slot of row r-1 is a
            # cheap bf16 SBUF shift-copy of row r's dy=0 slot (DVE 2x mode)
            pv4 = pv[:].rearrange("p (a c) -> p a c", a=4)
            nc.scalar.copy(stages[r // 8][:, :, r % 8, 0, :], pv4)
            if r > 0:
                nc.vector.tensor_copy(
                    stages[(r - 1) // 8][:, :, (r - 1) % 8, 1, :],
                    stages[r // 8][:, :, r % 8, 0, :])
            for _ in range(1 + r % 2):  # interleave weight-math ops
                next(wmg, None)
            if r % 8 == 0 and r > 0:
                b = r // 8 - 1
                flush(b, stages.pop(b))
        for _ in wmg:  # drain remaining weight-math ops
            pass
        nc.vector.memset(stages[15][:, :, 7, 1, :], 0.0)
        flush(15, stages.pop(15))

        # ---------- 4 & 5. gather (prepared early) + blend ----------
        sampled = cpool.tile([128, 8, 8, 32], F32)  # [q%128, qtop, h, c]
        bterm = cpool.tile([128, 8, 4, 32], F32)
        sT = [cpool.tile([128, 8, 128], F32R, name=f"sTi{i}", tag=f"sTi{i}")
              for i in range(2)]

        def half_tail(hf):
            # bias fold for heads 4hf..4hf+3, then transpose into sT[hf]
            sl = sampled[:, :, hf * 4:(hf + 1) * 4, :]
            nc.vector.tensor_tensor(
                bterm[:],
                sfac[:, :, hf * 4:(hf + 1) * 4].unsqueeze(3).broadcast_to(
                    [128, 8, 4, 32]),
                b_val[:].rearrange("p (h c) -> p h c", h=8)[
                    :, hf * 4:(hf + 1) * 4, :].unsqueeze(1).broadcast_to(
                    [128, 8, 4, 32]),
                TT.mult)
            nc.vector.tensor_tensor(sl, sl, bterm[:], TT.add)
            for qt_ in range(8):
                pt_ = ps_tr()
                nc.tensor.transpose(
                    pt_[:], sl[:, qt_].rearrange("p h c -> p (h c)"), ident[:])
                nc.scalar.copy(sT[hf][:, qt_, :], pt_[:])

        val_flat = value_d[:].rearrange("pr r c -> (pr r c)")
        QSCHED = (1, 2, 3, 0)
        for callid in range(32):
            h, pt_i = callid // 4, callid % 4
            half = h % 2
            pr = h // 2
            base = pr * ((ENT + EPAD) * 128)
            in_ap = val_flat[base:base + ENT * 128].rearrange(
                "(n c) -> n c", c=128).copy()
            in_ap.ap[-1] = (1, 256)  # overlapping 256-elem windows, step 128
            q = QSCHED[callid % 4]
            gat = gpool.tile([128, 8, 256], BF16, tag=f"gat{callid % 8}")
            nc.gpsimd.dma_gather(
                gat[:], in_ap,
                idxr[:, callid // 2,
                     (callid % 2) * 64:(callid % 2) * 64 + 64],
                LQ, LQ, 256, elem_step=128, queue_num=q)
            # window lanes: xj*128 + dy*64 + half*32 + c
            gv = gat[:].rearrange("p g (xj dy hl c) -> p g xj dy hl c",
                                  xj=2, dy=2, hl=2)
            cf = [P4c[:, xj, :, h, pt_i, :].rearrange(
                      "p d g -> p g d").unsqueeze(3).broadcast_to(
                      [128, 8, 2, 32]) for xj in range(2)]
            s0 = bpool.tile([128, 8, 2, 32], F32, tag="s0")
            s1 = bpool.tile([128, 8, 2, 32], F32, tag="s1")
            nc.vector.tensor_tensor(s0[:], gv[:, :, 0, :, half, :], cf[0],
                                    TT.mult)
            nc.vector.tensor_tensor(s1[:], gv[:, :, 1, :, half, :], cf[1],
                                    TT.mult)
            nc.vector.tensor_tensor(s0[:], s0[:], s1[:], TT.add)
            if pt_i == 0:
                nc.vector.tensor_tensor(sampled[:, :, h, :], s0[:, :, 0, :],
                                        s0[:, :, 1, :], TT.add)
            else:
                u = bpool.tile([128, 8, 32], F32, tag="u")
                nc.vector.tensor_tensor(u[:], s0[:, :, 0, :], s0[:, :, 1, :],
                                        TT.add)
                nc.vector.tensor_tensor(sampled[:, :, h, :],
                                        sampled[:, :, h, :], u[:], TT.add)
            if h in (3, 7) and pt_i == 3:
                half_tail(h // 4)

        # ---------- 6. output projection (sT halves emitted per h-half) ----
        for qt_ in range(8):
            po = ps_mm()
            for kc in range(2):
                nc.tensor.matmul(po[:], sT[kc][:, qt_, :], w_out[:, kc, :],
                                 start=(kc == 0), stop=(kc == 1))
            ot = qpool.tile([128, C], F32, tag="out")
            nc.vector.tensor_tensor(ot[:], po[:], b_out[:], TT.add)
            nc.sync.dma_start(out_e[qt_ * 128:(qt_ + 1) * 128, :], ot[:])

    nc.finalize()
    return nc


_CACHE = {}


def _get_program():
    if "nc" not in _CACHE:
        _CACHE["nc"] = build_program()
    return _CACHE["nc"]


def run(inputs, trace=False):
    from concourse.bass_utils import run_bass_kernel_spmd

    nc = _get_program()
    query = np.asarray(inputs["query"], np.float32)
    memory = np.asarray(inputs["memory"], np.float32)
    refpts = np.asarray(inputs["reference_points"], np.float32)
    w_value = np.asarray(inputs["W_value"], np.float32).astype(ml_dtypes.bfloat16)
    b_value = np.asarray(inputs["b_value"], np.float32)
    w_off = np.asarray(inputs["W_off"], np.float32)
    b_off = np.asarray(inputs["b_off"], np.float32)
    w_attn = np.asarray(inputs["W_attn"], np.float32)
    b_attn = np.asarray(inputs["b_attn"], np.float32)
    w_out = np.asarray(inputs["W_out"], np.float32)
    b_out = np.asarray(inputs["b_out"], np.float32)

    w_oa = np.concatenate([w_off, w_attn], axis=1).astype(np.float32)
    b_oa = np.tile(np.concatenate([b_off, b_attn])[None, :], (128, 1)).astype(
        np.float32)
    b_out_r = np.tile(b_out[None, :], (128, 1)).astype(np.float32)
    b_val_r = np.tile(b_value[None, :], (128, 1)).astype(np.float32)
    ident = np.eye(128, dtype=np.float32)

    shared = dict(w_value=w_value, w_oa=w_oa, b_oa=b_oa, w_out=w_out,
                  b_out=b_out_r, b_val=b_val_r, ident=ident)
    in_maps = []
    for i in range(N_B):
        m = dict(shared)
        m["qT"] = np.ascontiguousarray(query[i].T)
        m["memT"] = np.ascontiguousarray(memory[i].T).astype(ml_dtypes.bfloat16)
        m["refpts"] = refpts[i]
        in_maps.append(m)

    res = run_bass_kernel_spmd(nc, in_maps, list(range(N_B)), trace=trace,
                               trace_cores=[0])
    out = np.stack([res.results[i]["out"] for i in range(N_B)], axis=0)
    return out, res


def kernel(**inputs):
    assert int(inputs.get("H", GRID)) == GRID and int(inputs.get("W", GRID)) == GRID
    out, _ = run(inputs, trace=False)
    return out.astype(np.float32)
